# revision 1
# baseline (speedup 1.0000x reference)
"""Trainium2 Bass kernel for a pre-LN decoder block (attention + MLP).

Full-input contract: kernel(**inputs) takes the complete tensors
(x [64,512,384] fp32 + weights) and returns the full [64,512,384] output.
Internally: data-parallel over batch across 8 NeuronCores (8 batches per
core), weights replicated. No collectives needed.

Math per core (batch b, seq T=512, d=384, H=6 heads of 64):
  h  = LN(x) ;  qT/kT = W.T @ hT (heads packed 2-per-128-partitions)
  scoresT[s,t] = kT.T@qT (causal-trimmed, K=64 row-pairs packed)
  expT = exp(scoresT/sqrt(384)) (no max-sub: scores bounded ~|3|)
  attnT_aug = [v|1].T-style: lhsT=[v,1] so PSUM row 64 = softmax denom
  attnT = attnT_raw * bcast(1/denom) ; o = attnT.T@Wo + bo + x
  x2 = o ; h2 = LN(x2); mlp = relu(W1.T@h2T + bb1); out = mlp@W2 + bb2 + x2
Matmuls run in bf16 (fp32 PSUM accumulation); LN stats/softmax denom fp32.
"""

import math

import numpy as np

import concourse.bass as bass
import concourse.bacc as bacc
import concourse.mybir as mybir
import concourse.tile as tile
from concourse.bass_utils import run_bass_kernel_spmd
from concourse.masks import make_causal_mask  # noqa: F401  (pattern reference)

F32 = mybir.dt.float32
BF16 = mybir.dt.bfloat16

B, T, D = 64, 512, 384
H, HS = 6, 64
NCORES = 8
NB = B // NCORES            # 8 batches per core
FF = 4 * D                  # 1536
EPS = 1e-5
SCALE = 1.0 / math.sqrt(D)  # reference scales by sqrt(n_embd)
NT = T // 128               # 4 token tiles per batch
KD = D // 128               # 3 contraction tiles over model dim
KH = FF // 128              # 12 contraction tiles over hidden dim
NPAIR = H // 2              # 3 head pairs


def _emit(nc, tc, ctx, x, wq, wk, wv, wo, bo, w1, bb1, w2, bb2, out,
          nb=NB, dbg=None):
    """Emit the whole per-core program under TileContext tc."""
    P = 128

    # ---------------- pools ----------------
    wp = ctx.enter_context(tc.tile_pool(name="weights", bufs=1))
    xp = ctx.enter_context(tc.tile_pool(name="xres", bufs=8))
    hp = ctx.enter_context(tc.tile_pool(name="h", bufs=6))
    htp = ctx.enter_context(tc.tile_pool(name="hT", bufs=3))
    qkp = ctx.enter_context(tc.tile_pool(name="qk", bufs=6))
    vp = ctx.enter_context(tc.tile_pool(name="vaug", bufs=8))
    ep = ctx.enter_context(tc.tile_pool(name="expT", bufs=10))
    rp = ctx.enter_context(tc.tile_pool(name="rec", bufs=4))
    ap = ctx.enter_context(tc.tile_pool(name="attnT", bufs=3))
    mp = ctx.enter_context(tc.tile_pool(name="mlp", bufs=13))
    op = ctx.enter_context(tc.tile_pool(name="outp", bufs=6))
    sp = ctx.enter_context(tc.tile_pool(name="stats", bufs=4))

    pp512 = ctx.enter_context(tc.tile_pool(name="pp512", bufs=3, space="PSUM"))
    pp384 = ctx.enter_context(tc.tile_pool(name="pp384", bufs=2, space="PSUM"))
    ppat = ctx.enter_context(tc.tile_pool(name="ppat", bufs=2, space="PSUM"))
    ptrp = ctx.enter_context(tc.tile_pool(name="ptrp", bufs=1, space="PSUM"))

    # ---------------- constants + weights to SBUF (bf16) ----------------
    # wq_sb[kd]: [128, 384] cols h*64+e  (pair p occupies cols p*128..p*128+128)
    wq_sb, wk_sb, wv_sb = [], [], []
    for kd in range(KD):
        tq = wp.tile([P, H * HS], BF16, tag=f"wq{kd}", name=f"wq{kd}")
        tk = wp.tile([P, H * HS], BF16, tag=f"wk{kd}", name=f"wk{kd}")
        tv = wp.tile([P, H * HS], BF16, tag=f"wv{kd}", name=f"wv{kd}")
        for h in range(H):
            sl = slice(kd * P, (kd + 1) * P)
            nc.gpsimd.dma_start(out=tq[:, h * HS:(h + 1) * HS], in_=wq[h, sl, :])
            nc.gpsimd.dma_start(out=tk[:, h * HS:(h + 1) * HS], in_=wk[h, sl, :])
            nc.gpsimd.dma_start(out=tv[:, h * HS:(h + 1) * HS], in_=wv[h, sl, :])
        wq_sb.append(tq)
        wk_sb.append(tk)
        wv_sb.append(tv)

    wo_sb = []
    for ke in range(KD):
        t = wp.tile([P, D], BF16, tag=f"wo{ke}", name=f"wo{ke}")
        nc.gpsimd.dma_start(out=t, in_=wo[ke * P:(ke + 1) * P, :])
        wo_sb.append(t)

    w1_sb = []
    for kd in range(KD):
        t = wp.tile([P, FF], BF16, tag=f"w1{kd}", name=f"w1{kd}")
        nc.gpsimd.dma_start(out=t, in_=w1[kd * P:(kd + 1) * P, :])
        w1_sb.append(t)

    w2_sb = []
    for kh in range(KH):
        t = wp.tile([P, D], BF16, tag=f"w2{kh}", name=f"w2{kh}")
        nc.gpsimd.dma_start(out=t, in_=w2[kh * P:(kh + 1) * P, :])
        w2_sb.append(t)

    # biases: bb1 as per-partition scalars [128, 12]; bo/bb2 as [1, D] rows
    bb1_sb = wp.tile([P, KH], F32, tag="bb1", name="bb1")
    nc.sync.dma_start(out=bb1_sb, in_=bb1.rearrange("(a b) -> b a", b=P))
    bo_sb = wp.tile([1, D], BF16, tag="bo", name="bo")
    nc.gpsimd.dma_start(out=bo_sb, in_=bo.unsqueeze(0))
    bb2_sb = wp.tile([1, D], BF16, tag="bb2", name="bb2")
    nc.gpsimd.dma_start(out=bb2_sb, in_=bb2.unsqueeze(0))

    ones1 = wp.tile([1, P], BF16, tag="ones1", name="ones1")
    nc.vector.memset(ones1, 1.0)

    # upper-keep mask for the diagonal score block in [s,t] layout:
    # mask[s,t] = 1 if s <= t else 0
    masku = wp.tile([P, P], BF16, tag="masku", name="masku")
    nc.gpsimd.memset(masku, 1.0)
    nc.gpsimd.affine_select(
        out=masku, in_=masku,
        compare_op=mybir.AluOpType.is_ge,
        fill=0.0, base=0,
        pattern=[[1, P]], channel_multiplier=-1,
    )

    eps_sb = wp.tile([P, 1], F32, tag="eps", name="eps")
    nc.vector.memset(eps_sb, EPS)

    idn = wp.tile([P, P], BF16, tag="idn", name="idn")
    from concourse.masks import make_identity as _mkid
    _mkid(nc, idn)

    gamma_identity = True  # g=1, b=0 for this problem; keep general anyway
    del gamma_identity

    def layer_norm(x_tile, h_out):
        """x_tile [128, D] f32 -> h_out [128, D] bf16 (normalized, affine)."""
        stats = sp.tile([P, 6], F32, tag="bn_stats", name="bn_stats")
        nc.vector.bn_stats(out=stats, in_=x_tile)
        mv = sp.tile([P, 2], F32, tag="bn_aggr", name="bn_aggr")
        nc.vector.bn_aggr(out=mv, in_=stats)
        mean = mv[:, 0:1]
        var = mv[:, 1:2]
        # rstd = 1/sqrt(var+eps), with one Newton step to clean up ACT-sqrt +
        # fast-reciprocal error (~8e-3 worst -> ~1e-4)
        std = sp.tile([P, 1], F32, tag="ln_std", name="ln_std")
        nc.scalar.activation(out=std, in_=var,
                             func=mybir.ActivationFunctionType.Sqrt,
                             bias=eps_sb, scale=1.0)
        y0 = sp.tile([P, 1], F32, tag="ln_y0", name="ln_y0")
        nc.vector.reciprocal_approx_fast(out=y0, in_=std)
        ve = sp.tile([P, 1], F32, tag="ln_ve", name="ln_ve")
        nc.vector.tensor_scalar_add(out=ve, in0=var, scalar1=EPS)
        yy = sp.tile([P, 1], F32, tag="ln_yy", name="ln_yy")
        nc.vector.tensor_mul(out=yy, in0=y0, in1=y0)
        nc.vector.tensor_mul(out=yy, in0=yy, in1=ve)
        # yy <- 1.5 - 0.5*yy
        nc.vector.tensor_scalar(out=yy, in0=yy, scalar1=-0.5, scalar2=1.5,
                                op0=mybir.AluOpType.mult, op1=mybir.AluOpType.add)
        rstd = sp.tile([P, 1], F32, tag="ln_rstd", name="ln_rstd")
        nc.vector.tensor_mul(out=rstd, in0=y0, in1=yy)
        # normalized, cast to bf16 (gamma/beta pre-folded into the weights)
        nc.vector.tensor_scalar(out=h_out, in0=x_tile, scalar1=mean, scalar2=rstd,
                                op0=mybir.AluOpType.subtract,
                                op1=mybir.AluOpType.mult)

    # ---------------- per-batch pipeline ----------------
    for b in range(nb):
        # -- load x, LN1, transpose --
        x_tiles = []
        hT = [htp.tile([P, T], BF16, tag=f"hT{kd}", name=f"hT{kd}") for kd in range(KD)]
        for tt in range(NT):
            xt = xp.tile([P, D], F32, tag="x", name="x")
            nc.sync.dma_start(out=xt, in_=x[b, tt * P:(tt + 1) * P, :])
            x_tiles.append(xt)
            ht = hp.tile([P, D], BF16, tag="h1", name="h1")
            layer_norm(xt, ht)
            if dbg is not None and b == 0:
                nc.sync.dma_start(out=dbg["h1"][tt * P:(tt + 1) * P, :], in_=ht)
            pst = ptrp.tile([P, KD, P], BF16, tag="pst", name="pst")
            for kd in range(KD):
                nc.tensor.transpose(pst[:, kd, :], ht[:, kd * P:(kd + 1) * P], idn)
            for kd in range(KD):
                nc.vector.tensor_copy(out=hT[kd][:, tt * P:(tt + 1) * P],
                                      in_=pst[:, kd, :])

        # -- qT/kT (pairs, [128=2*64e, T]) and v_aug [128s, 6, 65] --
        qT, kT = [], []
        for p in range(NPAIR):
            for which, wsb, dst in (("q", wq_sb, qT), ("k", wk_sb, kT)):
                ps = pp512.tile([P, T], F32, tag="ps512", name="ps512")
                for kd in range(KD):
                    nc.tensor.matmul(ps, wsb[kd][:, p * P:(p + 1) * P], hT[kd],
                                     start=(kd == 0), stop=(kd == KD - 1))
                sb = qkp.tile([P, T], BF16, tag=f"{which}T", name=f"{which}T")
                nc.vector.tensor_copy(out=sb, in_=ps)
                if dbg is not None and b == 0:
                    nc.sync.dma_start(
                        out=dbg[f"{which}T"][p * P:(p + 1) * P, :], in_=sb)
                dst.append(sb)

        v_aug = []
        for ts in range(NT):
            ps = pp384.tile([P, D], F32, tag="ps384", name="ps384")
            for kd in range(KD):
                nc.tensor.matmul(ps, hT[kd][:, ts * P:(ts + 1) * P], wv_sb[kd],
                                 start=(kd == 0), stop=(kd == KD - 1))
            va = vp.tile([P, H, HS + 1], BF16, tag="vaug", name="vaug")
            nc.vector.memset(va[:, :, HS:HS + 1], 1.0)
            nc.vector.tensor_copy(
                out=va[:, :, 0:HS],
                in_=ps.rearrange("p (h e) -> p h e", h=H))
            if dbg is not None and b == 0:
                nc.sync.dma_start(
                    out=dbg["vaug"][ts * P:(ts + 1) * P, :],
                    in_=va.rearrange("p h e -> p (h e)"))
            v_aug.append(va)

        # -- attention per head --
        attnT = [ap.tile([P, T], BF16, tag=f"attnT{p}", name=f"attnT{p}") for p in range(NPAIR)]
        for p in range(NPAIR):
            for q in (0, 1):
                h = 2 * p + q
                esl = slice(q * HS, (q + 1) * HS)
                pat = ppat.tile([HS + 1, T], F32, tag="psattn", name="psattn")
                for ts in range(NT):
                    ncols = T - ts * P
                    tsl = slice(ts * P, T)
                    # scoresT[s, t] for s-tile ts, t >= ts*128
                    psc = pp512.tile([P, T], F32, tag="ps512", name="ps512")
                    nc.tensor.matmul(psc[:, 0:ncols],
                                     kT[p][esl, ts * P:(ts + 1) * P],
                                     qT[p][esl, tsl],
                                     start=True, stop=True)
                    et = ep.tile([P, T], BF16, tag="expT", name="expT")
                    nc.scalar.activation(out=et[:, 0:ncols], in_=psc[:, 0:ncols],
                                         func=mybir.ActivationFunctionType.Exp,
                                         scale=SCALE)
                    # mask diagonal block (cols 0:128 of this tile)
                    nc.vector.tensor_mul(out=et[:, 0:P], in0=et[:, 0:P], in1=masku)
                    if dbg is not None and b == 0 and h == 0:
                        nc.sync.dma_start(
                            out=dbg["expT"][ts * P:(ts + 1) * P, ts * P:T],
                            in_=et[:, 0:ncols])
                    # attnT_aug[e|1, t] += v_aug.T @ expT
                    nc.tensor.matmul(pat[:, tsl], v_aug[ts][:, h, :],
                                     et[:, 0:ncols],
                                     start=(ts == 0), stop=(ts == NT - 1),
                                     skip_group_check=True)
                # denom -> reciprocal -> broadcast over 64 partitions
                dnm = rp.tile([1, T], F32, tag="dnm", name="dnm")
                nc.vector.tensor_copy(out=dnm, in_=pat[HS:HS + 1, :])
                rrow = rp.tile([1, T], F32, tag="rrow", name="rrow")
                nc.vector.reciprocal_approx_fast(out=rrow, in_=dnm)
                if dbg is not None and b == 0 and h == 0:
                    nc.sync.dma_start(out=dbg["dnm"], in_=dnm)
                if dbg is not None and b == 0 and h == 0:
                    nc.sync.dma_start(out=dbg["rrow"], in_=rrow)
                rbc = rp.tile([HS, T], F32, tag="rbc", name="rbc")
                nc.gpsimd.partition_broadcast(out_ap=rbc, in_ap=rrow)
                nc.vector.tensor_mul(out=attnT[p][esl, :], in0=pat[0:HS, :],
                                     in1=rbc)
            if dbg is not None and b == 0:
                nc.sync.dma_start(out=dbg["attnT"][p * P:(p + 1) * P, :],
                                  in_=attnT[p])

        # -- out-proj + residual -> x2; LN2 -> h2T --
        x2_tiles = []
        h2T = [htp.tile([P, T], BF16, tag=f"h2T{kd}", name=f"h2T{kd}") for kd in range(KD)]
        for tt in range(NT):
            po = pp384.tile([P, D], F32, tag="ps384", name="ps384")
            for p in range(NPAIR):
                nc.tensor.matmul(po, attnT[p][:, tt * P:(tt + 1) * P], wo_sb[p],
                                 start=(p == 0), stop=False)
            nc.tensor.matmul(po, ones1, bo_sb, start=False, stop=True)
            x2t = xp.tile([P, D], F32, tag="x2", name="x2")
            nc.vector.tensor_add(out=x2t, in0=po, in1=x_tiles[tt])
            if dbg is not None and b == 0:
                nc.sync.dma_start(out=dbg["x2"][tt * P:(tt + 1) * P, :], in_=x2t)
            x2_tiles.append(x2t)
            h2 = hp.tile([P, D], BF16, tag="h2", name="h2")
            layer_norm(x2t, h2)
            pst2 = ptrp.tile([P, KD, P], BF16, tag="pst", name="pst2")
            for kd in range(KD):
                nc.tensor.transpose(pst2[:, kd, :], h2[:, kd * P:(kd + 1) * P], idn)
            for kd in range(KD):
                nc.vector.tensor_copy(out=h2T[kd][:, tt * P:(tt + 1) * P],
                                      in_=pst2[:, kd, :])

        # -- MLP --
        rT = []
        for kh in range(KH):
            pm = pp512.tile([P, T], F32, tag="ps512", name="ps512")
            for kd in range(KD):
                nc.tensor.matmul(pm, w1_sb[kd][:, kh * P:(kh + 1) * P], h2T[kd],
                                 start=(kd == 0), stop=(kd == KD - 1))
            rt = mp.tile([P, T], BF16, tag="rT", name="rT")
            nc.scalar.activation(out=rt, in_=pm,
                                 func=mybir.ActivationFunctionType.Relu,
                                 bias=bb1_sb[:, kh:kh + 1])
            if dbg is not None and b == 0 and kh == 0:
                nc.sync.dma_start(out=dbg["rT0"], in_=rt)
            rT.append(rt)

        for tt in range(NT):
            po2 = pp384.tile([P, D], F32, tag="ps384", name="ps384")
            for kh in range(KH):
                nc.tensor.matmul(po2, rT[kh][:, tt * P:(tt + 1) * P], w2_sb[kh],
                                 start=(kh == 0), stop=False)
            nc.tensor.matmul(po2, ones1, bb2_sb, start=False, stop=True)
            ot = op.tile([P, D], F32, tag="ot", name="ot")
            nc.vector.tensor_add(out=ot, in0=po2, in1=x2_tiles[tt])
            nc.sync.dma_start(out=out[b, tt * P:(tt + 1) * P, :], in_=ot)


def build(nb=NB, debug=False):
    from contextlib import ExitStack

    nc = bacc.Bacc("TRN2", target_bir_lowering=False, debug=False)
    P = 128
    x = nc.declare_dram_parameter("x", [nb, T, D], F32, isOutput=False).ap()
    wq = nc.declare_dram_parameter("Wq", [H, D, HS], F32, isOutput=False).ap()
    wk = nc.declare_dram_parameter("Wk", [H, D, HS], F32, isOutput=False).ap()
    wv = nc.declare_dram_parameter("Wv", [H, D, HS], F32, isOutput=False).ap()
    wo = nc.declare_dram_parameter("Wo", [D, D], F32, isOutput=False).ap()
    bo = nc.declare_dram_parameter("bo", [D], F32, isOutput=False).ap()
    w1 = nc.declare_dram_parameter("W1", [D, FF], F32, isOutput=False).ap()
    bb1 = nc.declare_dram_parameter("bb1", [FF], F32, isOutput=False).ap()
    w2 = nc.declare_dram_parameter("W2", [FF, D], F32, isOutput=False).ap()
    bb2 = nc.declare_dram_parameter("bb2", [D], F32, isOutput=False).ap()
    out = nc.declare_dram_parameter("out", [nb, T, D], F32, isOutput=True).ap()
    dbg = None
    if debug:
        dbg = {
            "h1": nc.declare_dram_parameter("dbg_h1", [T, D], BF16, isOutput=True).ap(),
            "qT": nc.declare_dram_parameter("dbg_qT", [D, T], BF16, isOutput=True).ap(),
            "kT": nc.declare_dram_parameter("dbg_kT", [D, T], BF16, isOutput=True).ap(),
            "vaug": nc.declare_dram_parameter("dbg_vaug", [T, H * (HS + 1)], BF16, isOutput=True).ap(),
            "expT": nc.declare_dram_parameter("dbg_expT", [T, T], BF16, isOutput=True).ap(),
            "rrow": nc.declare_dram_parameter("dbg_rrow", [1, T], F32, isOutput=True).ap(),
            "dnm": nc.declare_dram_parameter("dbg_dnm", [1, T], F32, isOutput=True).ap(),
            "attnT": nc.declare_dram_parameter("dbg_attnT", [D, T], BF16, isOutput=True).ap(),
            "x2": nc.declare_dram_parameter("dbg_x2", [T, D], F32, isOutput=True).ap(),
            "rT0": nc.declare_dram_parameter("dbg_rT0", [P, T], BF16, isOutput=True).ap(),
        }

    from contextlib import ExitStack as _ES
    with tile.TileContext(nc) as tc:
        with _ES() as ctx:
            _emit(nc, tc, ctx, x, wq, wk, wv, wo, bo,
                  w1, bb1, w2, bb2, out, nb=nb, dbg=dbg)
    nc.compile()
    return nc


def run(inputs, trace=False, **kw):
    x = np.asarray(inputs["x"], dtype=np.float32)
    nc = build()
    shared = {k: np.asarray(v, dtype=np.float32) for k, v in inputs.items()
              if k != "x"}
    # Fold LN affine params into the consuming weights (host-side, exact):
    #   h = LN0(x)*g + b ; h@W == LN0(x)@(g[:,None]*W) + b@W
    g1v, b1v = shared.pop("g1"), shared.pop("b1")
    g2v, b2v = shared.pop("g2"), shared.pop("b2")
    assert np.abs(b1v).max() == 0.0 and np.abs(b2v).max() == 0.0, \
        "nonzero LN beta not supported by this build"
    shared["Wq"] = shared["Wq"] * g1v[None, :, None]
    shared["Wk"] = shared["Wk"] * g1v[None, :, None]
    shared["Wv"] = shared["Wv"] * g1v[None, :, None]
    shared["W1"] = shared["W1"] * g2v[:, None]
    in_maps = []
    for c in range(NCORES):
        m = dict(shared)
        m["x"] = np.ascontiguousarray(x[c * NB:(c + 1) * NB])
        in_maps.append(m)
    res = run_bass_kernel_spmd(nc, in_maps, list(range(NCORES)), trace=trace, **kw)
    out = np.concatenate([r["out"] for r in res.results], axis=0)
    return out, res


def kernel(**inputs):
    return run(inputs)[0]


if __name__ == "__main__":
    rng = np.random.default_rng(0)
    ins = {
        "x": rng.standard_normal((B, T, D), dtype=np.float32),
    }
    print("built", build())



# revision 14
# speedup vs baseline: 1.2862x; 1.2862x over previous
"""Trainium2 Bass kernel for a pre-LN decoder block (attention + MLP).

Full-input contract: kernel(**inputs) takes the complete tensors
(x [64,512,384] fp32 + weights) and returns the full [64,512,384] output.
Internally: data-parallel over batch across 8 NeuronCores (8 batches per
core), weights replicated (host-packed, bf16). No collectives needed.

v2: software-pipelined across batches so the tensor engine never waits on
the LN / softmax vector chains:
  per iteration b:  LN2(b-1) -> attention(b) -> LN1(b+1) -> MLP(b-1)
                    -> Wo+residual(b)
Transposes go through the DMA xbar (dma_start_transpose, bf16), softmax
denominators are reciprocal'd straight from PSUM and broadcast with one
tensor-engine matmul per head pair, and PSUM->SBUF casts/relu run on the
scalar engine (closer to PSUM).
"""

import math

import numpy as np
import ml_dtypes

import concourse.bass as bass  # noqa: F401
import concourse.bacc as bacc
import concourse.mybir as mybir
import concourse.tile as tile
from concourse.bass_utils import run_bass_kernel_spmd

F32 = mybir.dt.float32
BF16 = mybir.dt.bfloat16

B, T, D = 64, 512, 384
H, HS = 6, 64
NCORES = 8
NB = B // NCORES            # 8 batches per core
FF = 4 * D                  # 1536
EPS = 1e-5
SCALE = 1.0 / math.sqrt(D)  # reference scales by sqrt(n_embd)
NT = T // 128               # 4 token tiles per batch
KD = D // 128               # 3 contraction tiles over model dim
KH = FF // 128              # 12 contraction tiles over hidden dim
NPAIR = H // 2              # 3 head pairs
P = 128


def _emit(nc, tc, ctx, x, wqkv, wo, w1, w2, bb1, out, nb=NB,
          zero_bias=True, bo=None, bb2=None, use_dma_transpose=True,
          dbg=None):
    # ---------------- pools ----------------
    wp = ctx.enter_context(tc.tile_pool(name="weights", bufs=1))
    xp = ctx.enter_context(tc.tile_pool(name="xres", bufs=12))
    x2p = ctx.enter_context(tc.tile_pool(name="x2res", bufs=8))
    hp = ctx.enter_context(tc.tile_pool(name="h", bufs=6))
    htp = ctx.enter_context(tc.tile_pool(name="hT", bufs=2))
    qkp = ctx.enter_context(tc.tile_pool(name="qk", bufs=2))
    vp = ctx.enter_context(tc.tile_pool(name="vaug", bufs=8))
    ep = ctx.enter_context(tc.tile_pool(name="expT", bufs=5))
    rp = ctx.enter_context(tc.tile_pool(name="rec", bufs=2))
    ap = ctx.enter_context(tc.tile_pool(name="attnT", bufs=2))
    mp = ctx.enter_context(tc.tile_pool(name="mlp", bufs=1))
    op = ctx.enter_context(tc.tile_pool(name="outp", bufs=3))
    sp = ctx.enter_context(tc.tile_pool(name="stats", bufs=2))

    pp512 = ctx.enter_context(tc.tile_pool(name="pp512", bufs=3, space="PSUM"))
    ppat = ctx.enter_context(tc.tile_pool(name="ppat", bufs=2, space="PSUM"))
    pp384 = ctx.enter_context(tc.tile_pool(name="pp384", bufs=3, space="PSUM"))
    if not use_dma_transpose:
        ptrp = ctx.enter_context(tc.tile_pool(name="ptrp", bufs=2, space="PSUM"))

    # ---------------- constants ----------------
    eps_sb = wp.tile([P, 1], F32, tag="eps", name="eps")
    nc.vector.memset(eps_sb, EPS)

    # upper-keep mask for the diagonal score block in [s,t] layout:
    # mask[s,t] = 1 if s <= t else 0
    masku = wp.tile([P, P], BF16, tag="masku", name="masku")
    nc.gpsimd.memset(masku, 1.0)
    nc.gpsimd.affine_select(
        out=masku, in_=masku,
        compare_op=mybir.AluOpType.is_ge,
        fill=0.0, base=0,
        pattern=[[1, P]], channel_multiplier=-1,
    )

    if not use_dma_transpose:
        from concourse.masks import make_identity as _mkid
        idn = wp.tile([P, P], BF16, tag="idn", name="idn")
        _mkid(nc, idn)

    # ---------------- weights -> SBUF (pre-packed bf16, one DMA each) -----
    wqkv_sb = [[wp.tile([P, H * HS], BF16, tag=f"wqkv{i}_{kd}",
                        name=f"wqkv{i}_{kd}") for kd in range(KD)]
               for i in range(3)]
    for i in range(3):
        for kd in range(KD):
            nc.sync.dma_start(out=wqkv_sb[i][kd], in_=wqkv[i, kd])
    wo_sb = [wp.tile([P, D], BF16, tag=f"wo{p}", name=f"wo{p}")
             for p in range(NPAIR)]
    for p in range(NPAIR):
        nc.sync.dma_start(out=wo_sb[p], in_=wo[p])
    w1_sb = [wp.tile([P, FF], BF16, tag=f"w1{kd}", name=f"w1{kd}")
             for kd in range(KD)]
    for kd in range(KD):
        nc.sync.dma_start(out=w1_sb[kd], in_=w1[kd])
    w2_sb = [wp.tile([P, D], BF16, tag=f"w2{kh}", name=f"w2{kh}")
             for kh in range(KH)]
    for kh in range(KH):
        nc.sync.dma_start(out=w2_sb[kh], in_=w2[kh])
    bb1_sb = wp.tile([P, KH], F32, tag="bb1", name="bb1")
    nc.scalar.dma_start(out=bb1_sb, in_=bb1)
    if not zero_bias:
        ones1 = wp.tile([1, P], BF16, tag="ones1", name="ones1")
        nc.vector.memset(ones1, 1.0)
        bo_sb = wp.tile([1, D], BF16, tag="bo", name="bo")
        nc.scalar.dma_start(out=bo_sb, in_=bo)
        bb2_sb = wp.tile([1, D], BF16, tag="bb2", name="bb2")
        nc.scalar.dma_start(out=bb2_sb, in_=bb2)

    # ---------------- helpers ----------------
    def layer_norm4(x4, h4, pfx):
        """x4: 4 tiles [128, D] f32 -> h4: 4 tiles [128, D] bf16 normalized.

        Stats batched across the 4 token tiles so the sqrt/reciprocal/Newton
        chain runs once on [128, 4] instead of per-tile."""
        stats = sp.tile([P, NT, 6], F32, tag=f"{pfx}_stats", name=f"{pfx}_stats")
        mv = sp.tile([P, NT, 2], F32, tag=f"{pfx}_mv", name=f"{pfx}_mv")
        for tt in range(NT):
            nc.vector.bn_stats(out=stats[:, tt, :], in_=x4[tt])
            nc.vector.bn_aggr(out=mv[:, tt, :], in_=stats[:, tt, :])
        std = sp.tile([P, NT], F32, tag=f"{pfx}_std", name=f"{pfx}_std")
        nc.scalar.activation(out=std, in_=mv[:, :, 1],
                             func=mybir.ActivationFunctionType.Sqrt,
                             bias=eps_sb, scale=1.0)
        y0 = sp.tile([P, NT], F32, tag=f"{pfx}_y0", name=f"{pfx}_y0")
        nc.vector.reciprocal_approx_fast(out=y0, in_=std)
        # one Newton step on rstd to clean up the ACT-sqrt table error
        ve = sp.tile([P, NT], F32, tag=f"{pfx}_ve", name=f"{pfx}_ve")
        nc.vector.tensor_scalar_add(out=ve, in0=mv[:, :, 1], scalar1=EPS)
        yy = sp.tile([P, NT], F32, tag=f"{pfx}_yy", name=f"{pfx}_yy")
        nc.vector.tensor_mul(out=yy, in0=y0, in1=y0)
        nc.vector.tensor_mul(out=yy, in0=yy, in1=ve)
        nc.vector.tensor_scalar(out=yy, in0=yy, scalar1=-0.5, scalar2=1.5,
                                op0=mybir.AluOpType.mult,
                                op1=mybir.AluOpType.add)
        rstd = sp.tile([P, NT], F32, tag=f"{pfx}_rstd", name=f"{pfx}_rstd")
        nc.vector.tensor_mul(out=rstd, in0=y0, in1=yy)
        for tt in range(NT):
            nc.vector.tensor_scalar(out=h4[tt], in0=x4[tt],
                                    scalar1=mv[:, tt, 0:1],
                                    scalar2=rstd[:, tt:tt + 1],
                                    op0=mybir.AluOpType.subtract,
                                    op1=mybir.AluOpType.mult)

    def transpose_h(h4, hT3, pfx):
        """h4: 4 tiles [128(t), D] bf16 -> hT3: 3 tiles [128(d), T] bf16."""
        if use_dma_transpose:
            for tt in range(NT):
                for kd in range(KD):
                    nc.sync.dma_start_transpose(
                        out=hT3[kd][:, tt * P:(tt + 1) * P],
                        in_=h4[tt][:, kd * P:(kd + 1) * P])
        else:
            for tt in range(NT):
                pst = ptrp.tile([P, KD, P], BF16, tag=f"pst", name=f"pst")
                for kd in range(KD):
                    nc.tensor.transpose(pst[:, kd, :],
                                        h4[tt][:, kd * P:(kd + 1) * P], idn)
                for kd in range(KD):
                    nc.vector.tensor_copy(
                        out=hT3[kd][:, tt * P:(tt + 1) * P], in_=pst[:, kd, :])

    x_tiles = {}
    x2_tiles = {}
    hT = {}
    h2T = {}
    attnT = {}

    def load_x(b):
        x_tiles[b] = []
        for tt in range(NT):
            xt = xp.tile([P, D], F32, tag="x", name="x")
            nc.sync.dma_start(out=xt, in_=x[b, tt * P:(tt + 1) * P, :])
            x_tiles[b].append(xt)

    def pre(b):
        """LN1(b) + transpose -> hT(b)."""
        h4 = [hp.tile([P, D], BF16, tag="h1", name="h1") for _ in range(NT)]
        layer_norm4(x_tiles[b], h4, "ln1")
        hT[b] = [htp.tile([P, T], BF16, tag=f"hT{kd}", name=f"hT{kd}")
                 for kd in range(KD)]
        transpose_h(h4, hT[b], "h1")
        if dbg is not None and b == 0:
            for tt in range(NT):
                nc.sync.dma_start(out=dbg["h1"][tt * P:(tt + 1) * P, :],
                                  in_=h4[tt])
            for kd in range(KD):
                nc.sync.dma_start(out=dbg["hT"][kd * P:(kd + 1) * P, :],
                                  in_=hT[0][kd])

    def mlp_front(b):
        """LN2(b) + transpose -> h2T(b)."""
        h4 = [hp.tile([P, D], BF16, tag="h2", name="h2") for _ in range(NT)]
        layer_norm4(x2_tiles[b], h4, "ln2")
        h2T[b] = [htp.tile([P, T], BF16, tag=f"h2T{kd}", name=f"h2T{kd}")
                  for kd in range(KD)]
        transpose_h(h4, h2T[b], "h2")

    def att_head(b):
        """qkv + scores/softmax/attnV + pair-normalize -> attnT(b)."""
        hTb = hT[b]
        qT, kT = [], []
        for p in range(NPAIR):
            for wofs, dst, nmq in ((0, qT, "qT"), (1, kT, "kT")):
                ps = pp512.tile([P, T], F32, tag="p512", name="ps512")
                for kd in range(KD):
                    nc.tensor.matmul(ps, wqkv_sb[wofs][kd][:, p * P:(p + 1) * P],
                                     hTb[kd], start=(kd == 0),
                                     stop=(kd == KD - 1))
                sb = qkp.tile([P, T], BF16, tag=f"{nmq}{p}", name=f"{nmq}{p}")
                nc.scalar.copy(out=sb, in_=ps)
                if dbg is not None and b == 0:
                    nc.sync.dma_start(out=dbg[nmq][p * P:(p + 1) * P, :], in_=sb)
                dst.append(sb)

        v_aug = []
        for ts in range(NT):
            ps = pp384.tile([P, D], F32, tag="p384", name="ps384",
                            padded_shape=[P, 512])
            for kd in range(KD):
                nc.tensor.matmul(ps, hTb[kd][:, ts * P:(ts + 1) * P],
                                 wqkv_sb[2][kd], start=(kd == 0),
                                 stop=(kd == KD - 1))
            va = vp.tile([P, H, HS + 1], BF16, tag="vaug", name="vaug")
            nc.vector.memset(va[:, :, HS:HS + 1], 1.0)
            nc.scalar.copy(out=va[:, :, 0:HS],
                           in_=ps.rearrange("p (h e) -> p h e", h=H))
            v_aug.append(va)
            if dbg is not None and b == 0:
                nc.sync.dma_start(
                    out=dbg["vaug"][ts * P:(ts + 1) * P, :],
                    in_=va.rearrange("p h e -> p (h e)"))

        attnT[b] = [ap.tile([P, T], BF16, tag=f"attnT{p}", name=f"attnT{p}")
                    for p in range(NPAIR)]
        for p in range(NPAIR):
            pats = []
            for q in (0, 1):
                h = 2 * p + q
                esl = slice(q * HS, (q + 1) * HS)
                pat = ppat.tile([HS + 1, T], F32, tag="pat", name="pat")
                pats.append(pat)
                # emit scores two tiles ahead of attnV so exp/mask latency
                # hides behind the next scores matmul
                ets = []

                def emit_sc(ts):
                    ncols = T - ts * P
                    psc = pp512.tile([P, T], F32, tag="p512", name="ps512")
                    nc.tensor.matmul(psc[:, 0:ncols],
                                     kT[p][esl, ts * P:(ts + 1) * P],
                                     qT[p][esl, ts * P:T],
                                     start=True, stop=True)
                    et = ep.tile([P, T], BF16, tag="expT", name="expT")
                    nc.scalar.activation(out=et[:, 0:ncols], in_=psc[:, 0:ncols],
                                         func=mybir.ActivationFunctionType.Exp,
                                         scale=SCALE)
                    nc.vector.tensor_mul(out=et[:, 0:P], in0=et[:, 0:P],
                                         in1=masku)
                    ets.append(et)

                def emit_av(ts):
                    ncols = T - ts * P
                    nc.tensor.matmul(pat[:, ts * P:T], v_aug[ts][:, h, :],
                                     ets[ts][:, 0:ncols],
                                     start=(ts == 0), stop=(ts == NT - 1),
                                     skip_group_check=True)

                emit_sc(0)
                emit_sc(1)
                emit_av(0)
                emit_sc(2)
                emit_av(1)
                emit_sc(3)
                emit_av(2)
                emit_av(3)
            # normalize the pair: 1/denom from the PSUM aug row, broadcast
            # across 64 partitions on the (otherwise idle) gpsimd engine,
            # then scale the raw attn rows
            for q in (0, 1):
                # custom-DVE reciprocal can't read PSUM: stage the aug row
                # through SBUF on the scalar engine (near PSUM, has slack)
                dnm = rp.tile([1, T], F32, tag=f"dnm{q}", name=f"dnm{q}")
                nc.scalar.copy(out=dnm, in_=pats[q][HS:HS + 1, :])
                rrh = rp.tile([1, T], F32, tag=f"rrh{q}", name=f"rrh{q}")
                nc.vector.reciprocal_approx_fast(out=rrh, in_=dnm)
                if dbg is not None and b == 0 and p == 0 and q == 0:
                    nc.sync.dma_start(out=dbg["dnm"], in_=rrh)
                rbc = rp.tile([HS, T], F32, tag=f"rbc{q}", name=f"rbc{q}")
                nc.gpsimd.partition_broadcast(out_ap=rbc, in_ap=rrh)
                nc.vector.tensor_mul(out=attnT[b][p][q * HS:(q + 1) * HS, :],
                                     in0=pats[q][0:HS, :], in1=rbc)
            if dbg is not None and b == 0:
                nc.sync.dma_start(out=dbg["attnT"][p * P:(p + 1) * P, :],
                                  in_=attnT[b][p])

    def att_tail(b):
        """Wo + residual -> x2(b)."""
        x2_tiles[b] = []
        for tt in range(NT):
            po = pp384.tile([P, D], F32, tag="p384", name="ps384",
                            padded_shape=[P, 512])
            for p in range(NPAIR):
                nc.tensor.matmul(po, attnT[b][p][:, tt * P:(tt + 1) * P],
                                 wo_sb[p], start=(p == 0),
                                 stop=(p == NPAIR - 1 and zero_bias))
            if not zero_bias:
                nc.tensor.matmul(po, ones1, bo_sb, start=False, stop=True)
            x2t = x2p.tile([P, D], F32, tag="x2", name="x2")
            nc.vector.tensor_add(out=x2t, in0=po, in1=x_tiles[b][tt])
            x2_tiles[b].append(x2t)

    def mlp_back(b):
        """W1 + relu + W2 + residual + store."""
        rT = []
        for kh in range(KH):
            pm = pp512.tile([P, T], F32, tag="p512", name="ps512")
            for kd in range(KD):
                nc.tensor.matmul(pm, w1_sb[kd][:, kh * P:(kh + 1) * P],
                                 h2T[b][kd], start=(kd == 0),
                                 stop=(kd == KD - 1))
            rt = mp.tile([P, T], BF16, tag=f"rT{kh}", name=f"rT{kh}")
            nc.scalar.activation(out=rt, in_=pm,
                                 func=mybir.ActivationFunctionType.Relu,
                                 bias=bb1_sb[:, kh:kh + 1])
            rT.append(rt)
        for tt in range(NT):
            po2 = pp384.tile([P, D], F32, tag="p384", name="ps384",
                             padded_shape=[P, 512])
            for kh in range(KH):
                nc.tensor.matmul(po2, rT[kh][:, tt * P:(tt + 1) * P],
                                 w2_sb[kh], start=(kh == 0),
                                 stop=(kh == KH - 1 and zero_bias))
            if not zero_bias:
                nc.tensor.matmul(po2, ones1, bb2_sb, start=False, stop=True)
            ot = op.tile([P, D], F32, tag="ot", name="ot")
            nc.vector.tensor_add(out=ot, in0=po2, in1=x2_tiles[b][tt])
            nc.scalar.dma_start(out=out[b, tt * P:(tt + 1) * P, :], in_=ot)

    # ---------------- pipelined schedule ----------------
    load_x(0)
    load_x(1)
    pre(0)
    for b in range(nb):
        if b >= 1:
            mlp_front(b - 1)
        att_head(b)
        if b + 2 < nb:
            load_x(b + 2)
        if b + 1 < nb:
            pre(b + 1)
        if b >= 1:
            mlp_back(b - 1)
        att_tail(b)
    mlp_front(nb - 1)
    mlp_back(nb - 1)


def build(nb=NB, zero_bias=True, use_dma_transpose=True, debug=False):
    from contextlib import ExitStack

    nc = bacc.Bacc("TRN2", target_bir_lowering=False, debug=False)
    x = nc.declare_dram_parameter("x", [nb, T, D], F32, isOutput=False).ap()
    wqkv = nc.declare_dram_parameter("wqkv", [3, KD, P, H * HS], BF16,
                                     isOutput=False).ap()
    wo = nc.declare_dram_parameter("wo", [KD, P, D], BF16, isOutput=False).ap()
    w1 = nc.declare_dram_parameter("w1", [KD, P, FF], BF16, isOutput=False).ap()
    w2 = nc.declare_dram_parameter("w2", [KH, P, D], BF16, isOutput=False).ap()
    bb1 = nc.declare_dram_parameter("bb1", [P, KH], F32, isOutput=False).ap()
    bo = bb2 = None
    if not zero_bias:
        bo = nc.declare_dram_parameter("bo", [1, D], BF16, isOutput=False).ap()
        bb2 = nc.declare_dram_parameter("bb2", [1, D], BF16, isOutput=False).ap()
    out = nc.declare_dram_parameter("out", [nb, T, D], F32, isOutput=True).ap()
    dbg = None
    if debug:
        dbg = {
            "h1": nc.declare_dram_parameter("dbg_h1", [T, D], BF16, isOutput=True).ap(),
            "hT": nc.declare_dram_parameter("dbg_hT", [D, T], BF16, isOutput=True).ap(),
            "qT": nc.declare_dram_parameter("dbg_qT", [D, T], BF16, isOutput=True).ap(),
            "kT": nc.declare_dram_parameter("dbg_kT", [D, T], BF16, isOutput=True).ap(),
            "vaug": nc.declare_dram_parameter("dbg_vaug", [T, H * (HS + 1)], BF16, isOutput=True).ap(),
            "dnm": nc.declare_dram_parameter("dbg_dnm", [1, T], F32, isOutput=True).ap(),
            "attnT": nc.declare_dram_parameter("dbg_attnT", [D, T], BF16, isOutput=True).ap(),
        }

    with tile.TileContext(nc) as tc:
        with ExitStack() as ctx:
            _emit(nc, tc, ctx, x, wqkv, wo, w1, w2, bb1, out, nb=nb,
                  zero_bias=zero_bias, bo=bo, bb2=bb2,
                  use_dma_transpose=use_dma_transpose, dbg=dbg)
    nc.compile()
    return nc


def _pack_qkv(w, g1):
    # [H, D, HS] * g1[d] -> [KD, 128, H*HS]
    w = w * g1[None, :, None]
    return w.transpose(1, 0, 2).reshape(D, H * HS).reshape(KD, P, H * HS)


def run(inputs, trace=False, use_dma_transpose=True, **kw):
    bf = ml_dtypes.bfloat16
    x = np.ascontiguousarray(np.asarray(inputs["x"], dtype=np.float32))
    g1 = np.asarray(inputs["g1"], np.float32)
    b1v = np.asarray(inputs["b1"], np.float32)
    g2 = np.asarray(inputs["g2"], np.float32)
    b2v = np.asarray(inputs["b2"], np.float32)
    assert np.abs(b1v).max() == 0.0 and np.abs(b2v).max() == 0.0, \
        "nonzero LN beta not supported by this build"
    bo = np.asarray(inputs["bo"], np.float32)
    bb2 = np.asarray(inputs["bb2"], np.float32)
    zero_bias = (np.abs(bo).max() == 0.0) and (np.abs(bb2).max() == 0.0)

    wqkv = np.stack([
        _pack_qkv(np.asarray(inputs["Wq"], np.float32), g1),
        _pack_qkv(np.asarray(inputs["Wk"], np.float32), g1),
        _pack_qkv(np.asarray(inputs["Wv"], np.float32), g1),
    ]).astype(bf)
    shared = {
        "wqkv": wqkv,
        "wo": np.asarray(inputs["Wo"], np.float32)
              .reshape(KD, P, D).astype(bf),
        "w1": (np.asarray(inputs["W1"], np.float32) * g2[:, None])
              .reshape(KD, P, FF).astype(bf),
        "w2": np.asarray(inputs["W2"], np.float32)
              .reshape(KH, P, D).astype(bf),
        "bb1": np.ascontiguousarray(
            np.asarray(inputs["bb1"], np.float32).reshape(KH, P).T),
    }
    if not zero_bias:
        shared["bo"] = bo.reshape(1, D).astype(bf)
        shared["bb2"] = bb2.reshape(1, D).astype(bf)

    nc = build(zero_bias=zero_bias, use_dma_transpose=use_dma_transpose)
    in_maps = []
    for c in range(NCORES):
        m = dict(shared)
        m["x"] = np.ascontiguousarray(x[c * NB:(c + 1) * NB])
        in_maps.append(m)
    res = run_bass_kernel_spmd(nc, in_maps, list(range(NCORES)), trace=trace,
                               **kw)
    outv = np.concatenate([r["out"] for r in res.results], axis=0)
    return outv, res


def kernel(**inputs):
    return run(inputs)[0]


if __name__ == "__main__":
    print("built", build())


# revision 15
# speedup vs baseline: 1.3145x; 1.0220x over previous
"""Trainium2 Bass kernel for a pre-LN decoder block (attention + MLP).

Full-input contract: kernel(**inputs) takes the complete tensors
(x [64,512,384] fp32 + weights) and returns the full [64,512,384] output.
Internally: data-parallel over batch across 8 NeuronCores (8 batches per
core), weights replicated (host-packed, bf16). No collectives needed.

v2: software-pipelined across batches so the tensor engine never waits on
the LN / softmax vector chains:
  per iteration b:  LN2(b-1) -> attention(b) -> LN1(b+1) -> MLP(b-1)
                    -> Wo+residual(b)
Transposes go through the DMA xbar (dma_start_transpose, bf16), softmax
denominators are reciprocal'd straight from PSUM and broadcast with one
tensor-engine matmul per head pair, and PSUM->SBUF casts/relu run on the
scalar engine (closer to PSUM).
"""

import math

import numpy as np
import ml_dtypes

import concourse.bass as bass  # noqa: F401
import concourse.bacc as bacc
import concourse.mybir as mybir
import concourse.tile as tile
from concourse.bass_utils import run_bass_kernel_spmd

F32 = mybir.dt.float32
BF16 = mybir.dt.bfloat16

B, T, D = 64, 512, 384
H, HS = 6, 64
NCORES = 8
NB = B // NCORES            # 8 batches per core
FF = 4 * D                  # 1536
EPS = 1e-5
SCALE = 1.0 / math.sqrt(D)  # reference scales by sqrt(n_embd)
NT = T // 128               # 4 token tiles per batch
KD = D // 128               # 3 contraction tiles over model dim
KH = FF // 128              # 12 contraction tiles over hidden dim
NPAIR = H // 2              # 3 head pairs
P = 128


def _emit(nc, tc, ctx, x, wqkv, wo, w1, w2, bb1, out, nb=NB,
          zero_bias=True, bo=None, bb2=None, use_dma_transpose=True,
          dbg=None):
    # ---------------- pools ----------------
    wp = ctx.enter_context(tc.tile_pool(name="weights", bufs=1))
    xp = ctx.enter_context(tc.tile_pool(name="xres", bufs=12))
    x2p = ctx.enter_context(tc.tile_pool(name="x2res", bufs=8))
    hp = ctx.enter_context(tc.tile_pool(name="h", bufs=6))
    htp = ctx.enter_context(tc.tile_pool(name="hT", bufs=2))
    qkp = ctx.enter_context(tc.tile_pool(name="qk", bufs=2))
    vp = ctx.enter_context(tc.tile_pool(name="vaug", bufs=8))
    ep = ctx.enter_context(tc.tile_pool(name="expT", bufs=5))
    rp = ctx.enter_context(tc.tile_pool(name="rec", bufs=2))
    ap = ctx.enter_context(tc.tile_pool(name="attnT", bufs=2))
    mp = ctx.enter_context(tc.tile_pool(name="mlp", bufs=1))
    op = ctx.enter_context(tc.tile_pool(name="outp", bufs=3))
    sp = ctx.enter_context(tc.tile_pool(name="stats", bufs=2))

    pp512 = ctx.enter_context(tc.tile_pool(name="pp512", bufs=3, space="PSUM"))
    ppat = ctx.enter_context(tc.tile_pool(name="ppat", bufs=2, space="PSUM"))
    pp384 = ctx.enter_context(tc.tile_pool(name="pp384", bufs=3, space="PSUM"))
    if not use_dma_transpose:
        ptrp = ctx.enter_context(tc.tile_pool(name="ptrp", bufs=2, space="PSUM"))

    # ---------------- constants ----------------
    eps_sb = wp.tile([P, 1], F32, tag="eps", name="eps")
    nc.vector.memset(eps_sb, EPS)

    # upper-keep mask for the diagonal score block in [s,t] layout:
    # mask[s,t] = 1 if s <= t else 0
    masku = wp.tile([P, P], BF16, tag="masku", name="masku")
    nc.gpsimd.memset(masku, 1.0)
    nc.gpsimd.affine_select(
        out=masku, in_=masku,
        compare_op=mybir.AluOpType.is_ge,
        fill=0.0, base=0,
        pattern=[[1, P]], channel_multiplier=-1,
    )

    if not use_dma_transpose:
        from concourse.masks import make_identity as _mkid
        idn = wp.tile([P, P], BF16, tag="idn", name="idn")
        _mkid(nc, idn)

    # ---------------- weights -> SBUF (pre-packed bf16, one DMA each) -----
    wqkv_sb = [[wp.tile([P, H * HS], BF16, tag=f"wqkv{i}_{kd}",
                        name=f"wqkv{i}_{kd}") for kd in range(KD)]
               for i in range(3)]
    for i in range(3):
        for kd in range(KD):
            nc.sync.dma_start(out=wqkv_sb[i][kd], in_=wqkv[i, kd])
    wo_sb = [wp.tile([P, D], BF16, tag=f"wo{p}", name=f"wo{p}")
             for p in range(NPAIR)]
    for p in range(NPAIR):
        nc.sync.dma_start(out=wo_sb[p], in_=wo[p])
    w1_sb = [wp.tile([P, FF], BF16, tag=f"w1{kd}", name=f"w1{kd}")
             for kd in range(KD)]
    for kd in range(KD):
        nc.sync.dma_start(out=w1_sb[kd], in_=w1[kd])
    w2_sb = [wp.tile([P, D], BF16, tag=f"w2{kh}", name=f"w2{kh}")
             for kh in range(KH)]
    for kh in range(KH):
        nc.sync.dma_start(out=w2_sb[kh], in_=w2[kh])
    bb1_sb = wp.tile([P, KH], F32, tag="bb1", name="bb1")
    nc.scalar.dma_start(out=bb1_sb, in_=bb1)
    if not zero_bias:
        ones1 = wp.tile([1, P], BF16, tag="ones1", name="ones1")
        nc.vector.memset(ones1, 1.0)
        bo_sb = wp.tile([1, D], BF16, tag="bo", name="bo")
        nc.scalar.dma_start(out=bo_sb, in_=bo)
        bb2_sb = wp.tile([1, D], BF16, tag="bb2", name="bb2")
        nc.scalar.dma_start(out=bb2_sb, in_=bb2)

    # ---------------- helpers ----------------
    def layer_norm4(x4, h4, pfx):
        """x4: 4 tiles [128, D] f32 -> h4: 4 tiles [128, D] bf16 normalized.

        Stats batched across the 4 token tiles so the sqrt/reciprocal/Newton
        chain runs once on [128, 4] instead of per-tile."""
        stats = sp.tile([P, NT, 6], F32, tag=f"{pfx}_stats", name=f"{pfx}_stats")
        mv = sp.tile([P, NT, 2], F32, tag=f"{pfx}_mv", name=f"{pfx}_mv")
        for tt in range(NT):
            nc.vector.bn_stats(out=stats[:, tt, :], in_=x4[tt])
            nc.vector.bn_aggr(out=mv[:, tt, :], in_=stats[:, tt, :])
        std = sp.tile([P, NT], F32, tag=f"{pfx}_std", name=f"{pfx}_std")
        nc.scalar.activation(out=std, in_=mv[:, :, 1],
                             func=mybir.ActivationFunctionType.Sqrt,
                             bias=eps_sb, scale=1.0)
        y0 = sp.tile([P, NT], F32, tag=f"{pfx}_y0", name=f"{pfx}_y0")
        nc.vector.reciprocal_approx_fast(out=y0, in_=std)
        # one Newton step on rstd to clean up the ACT-sqrt table error
        ve = sp.tile([P, NT], F32, tag=f"{pfx}_ve", name=f"{pfx}_ve")
        nc.vector.tensor_scalar_add(out=ve, in0=mv[:, :, 1], scalar1=EPS)
        yy = sp.tile([P, NT], F32, tag=f"{pfx}_yy", name=f"{pfx}_yy")
        nc.vector.tensor_mul(out=yy, in0=y0, in1=y0)
        nc.vector.tensor_mul(out=yy, in0=yy, in1=ve)
        nc.vector.tensor_scalar(out=yy, in0=yy, scalar1=-0.5, scalar2=1.5,
                                op0=mybir.AluOpType.mult,
                                op1=mybir.AluOpType.add)
        rstd = sp.tile([P, NT], F32, tag=f"{pfx}_rstd", name=f"{pfx}_rstd")
        nc.vector.tensor_mul(out=rstd, in0=y0, in1=yy)
        for tt in range(NT):
            nc.vector.tensor_scalar(out=h4[tt], in0=x4[tt],
                                    scalar1=mv[:, tt, 0:1],
                                    scalar2=rstd[:, tt:tt + 1],
                                    op0=mybir.AluOpType.subtract,
                                    op1=mybir.AluOpType.mult)

    def transpose_h(h4, hT3, pfx):
        """h4: 4 tiles [128(t), D] bf16 -> hT3: 3 tiles [128(d), T] bf16."""
        if use_dma_transpose:
            for tt in range(NT):
                for kd in range(KD):
                    nc.sync.dma_start_transpose(
                        out=hT3[kd][:, tt * P:(tt + 1) * P],
                        in_=h4[tt][:, kd * P:(kd + 1) * P])
        else:
            for tt in range(NT):
                pst = ptrp.tile([P, KD, P], BF16, tag=f"pst", name=f"pst")
                for kd in range(KD):
                    nc.tensor.transpose(pst[:, kd, :],
                                        h4[tt][:, kd * P:(kd + 1) * P], idn)
                for kd in range(KD):
                    nc.vector.tensor_copy(
                        out=hT3[kd][:, tt * P:(tt + 1) * P], in_=pst[:, kd, :])

    x_tiles = {}
    x2_tiles = {}
    hT = {}
    h2T = {}
    attnT = {}

    def load_x(b):
        x_tiles[b] = []
        for tt in range(NT):
            xt = xp.tile([P, D], F32, tag="x", name="x")
            nc.sync.dma_start(out=xt, in_=x[b, tt * P:(tt + 1) * P, :])
            x_tiles[b].append(xt)

    def pre(b):
        """LN1(b) + transpose -> hT(b)."""
        h4 = [hp.tile([P, D], BF16, tag="h1", name="h1") for _ in range(NT)]
        layer_norm4(x_tiles[b], h4, "ln1")
        hT[b] = [htp.tile([P, T], BF16, tag=f"hT{kd}", name=f"hT{kd}")
                 for kd in range(KD)]
        transpose_h(h4, hT[b], "h1")
        if dbg is not None and b == 0:
            for tt in range(NT):
                nc.sync.dma_start(out=dbg["h1"][tt * P:(tt + 1) * P, :],
                                  in_=h4[tt])
            for kd in range(KD):
                nc.sync.dma_start(out=dbg["hT"][kd * P:(kd + 1) * P, :],
                                  in_=hT[0][kd])

    def mlp_front(b):
        """LN2(b) + transpose -> h2T(b)."""
        h4 = [hp.tile([P, D], BF16, tag="h2", name="h2") for _ in range(NT)]
        layer_norm4(x2_tiles[b], h4, "ln2")
        h2T[b] = [htp.tile([P, T], BF16, tag=f"h2T{kd}", name=f"h2T{kd}")
                  for kd in range(KD)]
        transpose_h(h4, h2T[b], "h2")

    def att_head(b):
        """qkv + scores/softmax/attnV + pair-normalize -> attnT(b)."""
        hTb = hT[b]
        qT, kT = [], []
        for p in range(NPAIR):
            for wofs, dst, nmq in ((0, qT, "qT"), (1, kT, "kT")):
                ps = pp512.tile([P, T], F32, tag="p512", name="ps512")
                for kd in range(KD):
                    nc.tensor.matmul(ps, wqkv_sb[wofs][kd][:, p * P:(p + 1) * P],
                                     hTb[kd], start=(kd == 0),
                                     stop=(kd == KD - 1))
                sb = qkp.tile([P, T], BF16, tag=f"{nmq}{p}", name=f"{nmq}{p}")
                nc.scalar.copy(out=sb, in_=ps)
                if dbg is not None and b == 0:
                    nc.sync.dma_start(out=dbg[nmq][p * P:(p + 1) * P, :], in_=sb)
                dst.append(sb)

        v_aug = []
        for ts in range(NT):
            ps = pp384.tile([P, D], F32, tag="p384", name="ps384",
                            padded_shape=[P, 512])
            for kd in range(KD):
                nc.tensor.matmul(ps, hTb[kd][:, ts * P:(ts + 1) * P],
                                 wqkv_sb[2][kd], start=(kd == 0),
                                 stop=(kd == KD - 1))
            va = vp.tile([P, H, HS + 1], BF16, tag="vaug", name="vaug")
            nc.vector.memset(va[:, :, HS:HS + 1], 1.0)
            nc.scalar.copy(out=va[:, :, 0:HS],
                           in_=ps.rearrange("p (h e) -> p h e", h=H))
            v_aug.append(va)
            if dbg is not None and b == 0:
                nc.sync.dma_start(
                    out=dbg["vaug"][ts * P:(ts + 1) * P, :],
                    in_=va.rearrange("p h e -> p (h e)"))

        attnT[b] = [ap.tile([P, T], BF16, tag=f"attnT{p}", name=f"attnT{p}")
                    for p in range(NPAIR)]
        for p in range(NPAIR):
            # interleave the pair's two heads: their K=64 score matmuls sit
            # on disjoint 64-row halves of the PE array (tile_position rows
            # 0 / 64 auto-derived from the operand base partition), so
            # adjacent emission lets them execute concurrently
            pats = [ppat.tile([HS + 1, T], F32, tag="pat", name="pat")
                    for _ in (0, 1)]
            ets = ([], [])

            def emit_sc(q, ts):
                esl = slice(q * HS, (q + 1) * HS)
                ncols = T - ts * P
                psc = pp512.tile([P, T], F32, tag="p512", name="ps512")
                nc.tensor.matmul(psc[:, 0:ncols],
                                 kT[p][esl, ts * P:(ts + 1) * P],
                                 qT[p][esl, ts * P:T],
                                 start=True, stop=True)
                et = ep.tile([P, T], BF16, tag="expT", name="expT")
                nc.scalar.activation(out=et[:, 0:ncols], in_=psc[:, 0:ncols],
                                     func=mybir.ActivationFunctionType.Exp,
                                     scale=SCALE)
                nc.vector.tensor_mul(out=et[:, 0:P], in0=et[:, 0:P],
                                     in1=masku)
                ets[q].append(et)

            def emit_av(q, ts):
                ncols = T - ts * P
                nc.tensor.matmul(pats[q][:, ts * P:T],
                                 v_aug[ts][:, 2 * p + q, :],
                                 ets[q][ts][:, 0:ncols],
                                 start=(ts == 0), stop=(ts == NT - 1),
                                 skip_group_check=True)

            emit_sc(0, 0)
            emit_sc(1, 0)
            emit_sc(0, 1)
            emit_sc(1, 1)
            emit_av(0, 0)
            emit_av(1, 0)
            emit_sc(0, 2)
            emit_sc(1, 2)
            emit_av(0, 1)
            emit_av(1, 1)
            emit_sc(0, 3)
            emit_sc(1, 3)
            emit_av(0, 2)
            emit_av(1, 2)
            emit_av(0, 3)
            emit_av(1, 3)
            # normalize the pair: 1/denom from the PSUM aug row, broadcast
            # across 64 partitions on the (otherwise idle) gpsimd engine,
            # then scale the raw attn rows
            for q in (0, 1):
                # custom-DVE reciprocal can't read PSUM: stage the aug row
                # through SBUF on the scalar engine (near PSUM, has slack)
                dnm = rp.tile([1, T], F32, tag=f"dnm{q}", name=f"dnm{q}")
                nc.scalar.copy(out=dnm, in_=pats[q][HS:HS + 1, :])
                rrh = rp.tile([1, T], F32, tag=f"rrh{q}", name=f"rrh{q}")
                nc.vector.reciprocal_approx_fast(out=rrh, in_=dnm)
                if dbg is not None and b == 0 and p == 0 and q == 0:
                    nc.sync.dma_start(out=dbg["dnm"], in_=rrh)
                rbc = rp.tile([HS, T], F32, tag=f"rbc{q}", name=f"rbc{q}")
                nc.gpsimd.partition_broadcast(out_ap=rbc, in_ap=rrh)
                nc.vector.tensor_mul(out=attnT[b][p][q * HS:(q + 1) * HS, :],
                                     in0=pats[q][0:HS, :], in1=rbc)
            if dbg is not None and b == 0:
                nc.sync.dma_start(out=dbg["attnT"][p * P:(p + 1) * P, :],
                                  in_=attnT[b][p])

    def att_tail(b):
        """Wo + residual -> x2(b)."""
        x2_tiles[b] = []
        for tt in range(NT):
            po = pp384.tile([P, D], F32, tag="p384", name="ps384",
                            padded_shape=[P, 512])
            for p in range(NPAIR):
                nc.tensor.matmul(po, attnT[b][p][:, tt * P:(tt + 1) * P],
                                 wo_sb[p], start=(p == 0),
                                 stop=(p == NPAIR - 1 and zero_bias))
            if not zero_bias:
                nc.tensor.matmul(po, ones1, bo_sb, start=False, stop=True)
            x2t = x2p.tile([P, D], F32, tag="x2", name="x2")
            nc.vector.tensor_add(out=x2t, in0=po, in1=x_tiles[b][tt])
            x2_tiles[b].append(x2t)

    def mlp_back(b):
        """W1 + relu + W2 + residual + store."""
        rT = []
        for kh in range(KH):
            pm = pp512.tile([P, T], F32, tag="p512", name="ps512")
            for kd in range(KD):
                nc.tensor.matmul(pm, w1_sb[kd][:, kh * P:(kh + 1) * P],
                                 h2T[b][kd], start=(kd == 0),
                                 stop=(kd == KD - 1))
            rt = mp.tile([P, T], BF16, tag=f"rT{kh}", name=f"rT{kh}")
            nc.scalar.activation(out=rt, in_=pm,
                                 func=mybir.ActivationFunctionType.Relu,
                                 bias=bb1_sb[:, kh:kh + 1])
            rT.append(rt)
        for tt in range(NT):
            po2 = pp384.tile([P, D], F32, tag="p384", name="ps384",
                             padded_shape=[P, 512])
            for kh in range(KH):
                nc.tensor.matmul(po2, rT[kh][:, tt * P:(tt + 1) * P],
                                 w2_sb[kh], start=(kh == 0),
                                 stop=(kh == KH - 1 and zero_bias))
            if not zero_bias:
                nc.tensor.matmul(po2, ones1, bb2_sb, start=False, stop=True)
            ot = op.tile([P, D], F32, tag="ot", name="ot")
            nc.vector.tensor_add(out=ot, in0=po2, in1=x2_tiles[b][tt])
            nc.scalar.dma_start(out=out[b, tt * P:(tt + 1) * P, :], in_=ot)

    # ---------------- pipelined schedule ----------------
    load_x(0)
    load_x(1)
    pre(0)
    for b in range(nb):
        if b >= 1:
            mlp_front(b - 1)
        att_head(b)
        if b + 2 < nb:
            load_x(b + 2)
        if b + 1 < nb:
            pre(b + 1)
        if b >= 1:
            mlp_back(b - 1)
        att_tail(b)
    mlp_front(nb - 1)
    mlp_back(nb - 1)


def build(nb=NB, zero_bias=True, use_dma_transpose=True, debug=False):
    from contextlib import ExitStack

    nc = bacc.Bacc("TRN2", target_bir_lowering=False, debug=False)
    x = nc.declare_dram_parameter("x", [nb, T, D], F32, isOutput=False).ap()
    wqkv = nc.declare_dram_parameter("wqkv", [3, KD, P, H * HS], BF16,
                                     isOutput=False).ap()
    wo = nc.declare_dram_parameter("wo", [KD, P, D], BF16, isOutput=False).ap()
    w1 = nc.declare_dram_parameter("w1", [KD, P, FF], BF16, isOutput=False).ap()
    w2 = nc.declare_dram_parameter("w2", [KH, P, D], BF16, isOutput=False).ap()
    bb1 = nc.declare_dram_parameter("bb1", [P, KH], F32, isOutput=False).ap()
    bo = bb2 = None
    if not zero_bias:
        bo = nc.declare_dram_parameter("bo", [1, D], BF16, isOutput=False).ap()
        bb2 = nc.declare_dram_parameter("bb2", [1, D], BF16, isOutput=False).ap()
    out = nc.declare_dram_parameter("out", [nb, T, D], F32, isOutput=True).ap()
    dbg = None
    if debug:
        dbg = {
            "h1": nc.declare_dram_parameter("dbg_h1", [T, D], BF16, isOutput=True).ap(),
            "hT": nc.declare_dram_parameter("dbg_hT", [D, T], BF16, isOutput=True).ap(),
            "qT": nc.declare_dram_parameter("dbg_qT", [D, T], BF16, isOutput=True).ap(),
            "kT": nc.declare_dram_parameter("dbg_kT", [D, T], BF16, isOutput=True).ap(),
            "vaug": nc.declare_dram_parameter("dbg_vaug", [T, H * (HS + 1)], BF16, isOutput=True).ap(),
            "dnm": nc.declare_dram_parameter("dbg_dnm", [1, T], F32, isOutput=True).ap(),
            "attnT": nc.declare_dram_parameter("dbg_attnT", [D, T], BF16, isOutput=True).ap(),
        }

    with tile.TileContext(nc) as tc:
        with ExitStack() as ctx:
            _emit(nc, tc, ctx, x, wqkv, wo, w1, w2, bb1, out, nb=nb,
                  zero_bias=zero_bias, bo=bo, bb2=bb2,
                  use_dma_transpose=use_dma_transpose, dbg=dbg)
    nc.compile()
    return nc


def _pack_qkv(w, g1):
    # [H, D, HS] * g1[d] -> [KD, 128, H*HS]
    w = w * g1[None, :, None]
    return w.transpose(1, 0, 2).reshape(D, H * HS).reshape(KD, P, H * HS)


def run(inputs, trace=False, use_dma_transpose=True, **kw):
    bf = ml_dtypes.bfloat16
    x = np.ascontiguousarray(np.asarray(inputs["x"], dtype=np.float32))
    g1 = np.asarray(inputs["g1"], np.float32)
    b1v = np.asarray(inputs["b1"], np.float32)
    g2 = np.asarray(inputs["g2"], np.float32)
    b2v = np.asarray(inputs["b2"], np.float32)
    assert np.abs(b1v).max() == 0.0 and np.abs(b2v).max() == 0.0, \
        "nonzero LN beta not supported by this build"
    bo = np.asarray(inputs["bo"], np.float32)
    bb2 = np.asarray(inputs["bb2"], np.float32)
    zero_bias = (np.abs(bo).max() == 0.0) and (np.abs(bb2).max() == 0.0)

    wqkv = np.stack([
        _pack_qkv(np.asarray(inputs["Wq"], np.float32), g1),
        _pack_qkv(np.asarray(inputs["Wk"], np.float32), g1),
        _pack_qkv(np.asarray(inputs["Wv"], np.float32), g1),
    ]).astype(bf)
    shared = {
        "wqkv": wqkv,
        "wo": np.asarray(inputs["Wo"], np.float32)
              .reshape(KD, P, D).astype(bf),
        "w1": (np.asarray(inputs["W1"], np.float32) * g2[:, None])
              .reshape(KD, P, FF).astype(bf),
        "w2": np.asarray(inputs["W2"], np.float32)
              .reshape(KH, P, D).astype(bf),
        "bb1": np.ascontiguousarray(
            np.asarray(inputs["bb1"], np.float32).reshape(KH, P).T),
    }
    if not zero_bias:
        shared["bo"] = bo.reshape(1, D).astype(bf)
        shared["bb2"] = bb2.reshape(1, D).astype(bf)

    nc = build(zero_bias=zero_bias, use_dma_transpose=use_dma_transpose)
    in_maps = []
    for c in range(NCORES):
        m = dict(shared)
        m["x"] = np.ascontiguousarray(x[c * NB:(c + 1) * NB])
        in_maps.append(m)
    res = run_bass_kernel_spmd(nc, in_maps, list(range(NCORES)), trace=trace,
                               **kw)
    outv = np.concatenate([r["out"] for r in res.results], axis=0)
    return outv, res


def kernel(**inputs):
    return run(inputs)[0]


if __name__ == "__main__":
    print("built", build())


# revision 17
# speedup vs baseline: 1.3318x; 1.0131x over previous
"""Trainium2 Bass kernel for a pre-LN decoder block (attention + MLP).

Full-input contract: kernel(**inputs) takes the complete tensors
(x [64,512,384] fp32 + weights) and returns the full [64,512,384] output.
Internally: data-parallel over batch across 8 NeuronCores (8 batches per
core), weights replicated (host-packed, bf16). No collectives needed.

v2: software-pipelined across batches so the tensor engine never waits on
the LN / softmax vector chains:
  per iteration b:  LN2(b-1) -> attention(b) -> LN1(b+1) -> MLP(b-1)
                    -> Wo+residual(b)
Transposes go through the DMA xbar (dma_start_transpose, bf16), softmax
denominators are reciprocal'd straight from PSUM and broadcast with one
tensor-engine matmul per head pair, and PSUM->SBUF casts/relu run on the
scalar engine (closer to PSUM).
"""

import math

import numpy as np
import ml_dtypes

import concourse.bass as bass  # noqa: F401
import concourse.bacc as bacc
import concourse.mybir as mybir
import concourse.tile as tile
from concourse.bass_utils import run_bass_kernel_spmd

F32 = mybir.dt.float32
BF16 = mybir.dt.bfloat16

B, T, D = 64, 512, 384
H, HS = 6, 64
NCORES = 8
NB = B // NCORES            # 8 batches per core
FF = 4 * D                  # 1536
EPS = 1e-5
SCALE = 1.0 / math.sqrt(D)  # reference scales by sqrt(n_embd)
NT = T // 128               # 4 token tiles per batch
KD = D // 128               # 3 contraction tiles over model dim
KH = FF // 128              # 12 contraction tiles over hidden dim
NPAIR = H // 2              # 3 head pairs
P = 128


def _emit(nc, tc, ctx, x, wqkv, wo, w1, w2, bb1, out, nb=NB,
          zero_bias=True, bo=None, bb2=None, use_dma_transpose=True,
          dbg=None):
    # ---------------- pools ----------------
    wp = ctx.enter_context(tc.tile_pool(name="weights", bufs=1))
    xp = ctx.enter_context(tc.tile_pool(name="xres", bufs=12))
    x2p = ctx.enter_context(tc.tile_pool(name="x2res", bufs=8))
    hp = ctx.enter_context(tc.tile_pool(name="h", bufs=6))
    htp = ctx.enter_context(tc.tile_pool(name="hT", bufs=2))
    qkp = ctx.enter_context(tc.tile_pool(name="qk", bufs=2))
    vp = ctx.enter_context(tc.tile_pool(name="vaug", bufs=8))
    ep = ctx.enter_context(tc.tile_pool(name="expT", bufs=5))
    rp = ctx.enter_context(tc.tile_pool(name="rec", bufs=2))
    ap = ctx.enter_context(tc.tile_pool(name="attnT", bufs=2))
    mp = ctx.enter_context(tc.tile_pool(name="mlp", bufs=1))
    op = ctx.enter_context(tc.tile_pool(name="outp", bufs=3))
    sp = ctx.enter_context(tc.tile_pool(name="stats", bufs=2))

    pp512 = ctx.enter_context(tc.tile_pool(name="pp512", bufs=3, space="PSUM"))
    ppat = ctx.enter_context(tc.tile_pool(name="ppat", bufs=2, space="PSUM"))
    pp384 = ctx.enter_context(tc.tile_pool(name="pp384", bufs=3, space="PSUM"))
    if not use_dma_transpose:
        ptrp = ctx.enter_context(tc.tile_pool(name="ptrp", bufs=2, space="PSUM"))

    # ---------------- constants ----------------
    eps_sb = wp.tile([P, 1], F32, tag="eps", name="eps")
    nc.vector.memset(eps_sb, EPS)

    # upper-keep mask for the diagonal score block in [s,t] layout:
    # mask[s,t] = 1 if s <= t else 0
    masku = wp.tile([P, P], BF16, tag="masku", name="masku")
    nc.gpsimd.memset(masku, 1.0)
    nc.gpsimd.affine_select(
        out=masku, in_=masku,
        compare_op=mybir.AluOpType.is_ge,
        fill=0.0, base=0,
        pattern=[[1, P]], channel_multiplier=-1,
    )

    if not use_dma_transpose:
        from concourse.masks import make_identity as _mkid
        idn = wp.tile([P, P], BF16, tag="idn", name="idn")
        _mkid(nc, idn)

    # ---------------- weights -> SBUF (pre-packed bf16, one DMA each) -----
    wqkv_sb = [[wp.tile([P, H * HS], BF16, tag=f"wqkv{i}_{kd}",
                        name=f"wqkv{i}_{kd}") for kd in range(KD)]
               for i in range(3)]
    for i in range(3):
        for kd in range(KD):
            nc.sync.dma_start(out=wqkv_sb[i][kd], in_=wqkv[i, kd])
    wo_sb = [wp.tile([P, D], BF16, tag=f"wo{p}", name=f"wo{p}")
             for p in range(NPAIR)]
    for p in range(NPAIR):
        nc.sync.dma_start(out=wo_sb[p], in_=wo[p])
    w1_sb = [wp.tile([P, FF], BF16, tag=f"w1{kd}", name=f"w1{kd}")
             for kd in range(KD)]
    for kd in range(KD):
        nc.sync.dma_start(out=w1_sb[kd], in_=w1[kd])
    w2_sb = [wp.tile([P, D], BF16, tag=f"w2{kh}", name=f"w2{kh}")
             for kh in range(KH)]
    for kh in range(KH):
        nc.sync.dma_start(out=w2_sb[kh], in_=w2[kh])
    bb1_sb = wp.tile([P, KH], F32, tag="bb1", name="bb1")
    nc.scalar.dma_start(out=bb1_sb, in_=bb1)
    if not zero_bias:
        ones1 = wp.tile([1, P], BF16, tag="ones1", name="ones1")
        nc.vector.memset(ones1, 1.0)
        bo_sb = wp.tile([1, D], BF16, tag="bo", name="bo")
        nc.scalar.dma_start(out=bo_sb, in_=bo)
        bb2_sb = wp.tile([1, D], BF16, tag="bb2", name="bb2")
        nc.scalar.dma_start(out=bb2_sb, in_=bb2)

    # ---------------- helpers ----------------
    def layer_norm4(x4, h4, pfx):
        """x4: 4 tiles [128, D] f32 -> h4: 4 tiles [128, D] bf16 normalized.

        Stats batched across the 4 token tiles so the sqrt/reciprocal/Newton
        chain runs once on [128, 4] instead of per-tile."""
        stats = sp.tile([P, NT, 6], F32, tag=f"{pfx}_stats", name=f"{pfx}_stats")
        mv = sp.tile([P, NT, 2], F32, tag=f"{pfx}_mv", name=f"{pfx}_mv")
        for tt in range(NT):
            nc.vector.bn_stats(out=stats[:, tt, :], in_=x4[tt])
            nc.vector.bn_aggr(out=mv[:, tt, :], in_=stats[:, tt, :])
        std = sp.tile([P, NT], F32, tag=f"{pfx}_std", name=f"{pfx}_std")
        nc.scalar.activation(out=std, in_=mv[:, :, 1],
                             func=mybir.ActivationFunctionType.Sqrt,
                             bias=eps_sb, scale=1.0)
        y0 = sp.tile([P, NT], F32, tag=f"{pfx}_y0", name=f"{pfx}_y0")
        nc.vector.reciprocal_approx_fast(out=y0, in_=std)
        # one Newton step on rstd to clean up the ACT-sqrt table error
        ve = sp.tile([P, NT], F32, tag=f"{pfx}_ve", name=f"{pfx}_ve")
        nc.vector.tensor_scalar_add(out=ve, in0=mv[:, :, 1], scalar1=EPS)
        yy = sp.tile([P, NT], F32, tag=f"{pfx}_yy", name=f"{pfx}_yy")
        nc.vector.tensor_mul(out=yy, in0=y0, in1=y0)
        nc.vector.tensor_mul(out=yy, in0=yy, in1=ve)
        nc.vector.tensor_scalar(out=yy, in0=yy, scalar1=-0.5, scalar2=1.5,
                                op0=mybir.AluOpType.mult,
                                op1=mybir.AluOpType.add)
        rstd = sp.tile([P, NT], F32, tag=f"{pfx}_rstd", name=f"{pfx}_rstd")
        nc.vector.tensor_mul(out=rstd, in0=y0, in1=yy)
        for tt in range(NT):
            nc.vector.tensor_scalar(out=h4[tt], in0=x4[tt],
                                    scalar1=mv[:, tt, 0:1],
                                    scalar2=rstd[:, tt:tt + 1],
                                    op0=mybir.AluOpType.subtract,
                                    op1=mybir.AluOpType.mult)

    def transpose_h(h4, hT3, pfx):
        """h4: 4 tiles [128(t), D] bf16 -> hT3: 3 tiles [128(d), T] bf16."""
        if use_dma_transpose:
            for tt in range(NT):
                for kd in range(KD):
                    nc.sync.dma_start_transpose(
                        out=hT3[kd][:, tt * P:(tt + 1) * P],
                        in_=h4[tt][:, kd * P:(kd + 1) * P])
        else:
            for tt in range(NT):
                pst = ptrp.tile([P, KD, P], BF16, tag=f"pst", name=f"pst")
                for kd in range(KD):
                    nc.tensor.transpose(pst[:, kd, :],
                                        h4[tt][:, kd * P:(kd + 1) * P], idn)
                for kd in range(KD):
                    nc.vector.tensor_copy(
                        out=hT3[kd][:, tt * P:(tt + 1) * P], in_=pst[:, kd, :])

    x_tiles = {}
    x2_tiles = {}
    hT = {}
    h2T = {}
    attnT = {}

    def load_x(b):
        x_tiles[b] = []
        for tt in range(NT):
            xt = xp.tile([P, D], F32, tag="x", name="x")
            nc.sync.dma_start(out=xt, in_=x[b, tt * P:(tt + 1) * P, :])
            x_tiles[b].append(xt)

    def pre(b):
        """LN1(b) + transpose -> hT(b)."""
        h4 = [hp.tile([P, D], BF16, tag="h1", name="h1") for _ in range(NT)]
        layer_norm4(x_tiles[b], h4, "ln1")
        hT[b] = [htp.tile([P, T], BF16, tag=f"hT{kd}", name=f"hT{kd}")
                 for kd in range(KD)]
        transpose_h(h4, hT[b], "h1")
        if dbg is not None and b == 0:
            for tt in range(NT):
                nc.sync.dma_start(out=dbg["h1"][tt * P:(tt + 1) * P, :],
                                  in_=h4[tt])
            for kd in range(KD):
                nc.sync.dma_start(out=dbg["hT"][kd * P:(kd + 1) * P, :],
                                  in_=hT[0][kd])

    def mlp_front(b):
        """LN2(b) + transpose -> h2T(b)."""
        h4 = [hp.tile([P, D], BF16, tag="h2", name="h2") for _ in range(NT)]
        layer_norm4(x2_tiles[b], h4, "ln2")
        h2T[b] = [htp.tile([P, T], BF16, tag=f"h2T{kd}", name=f"h2T{kd}")
                  for kd in range(KD)]
        transpose_h(h4, h2T[b], "h2")

    def att_head(b):
        """qkv + scores/softmax/attnV + pair-normalize -> attnT(b)."""
        hTb = hT[b]
        qT, kT = [], []
        for p in range(NPAIR):
            for wofs, dst, nmq in ((0, qT, "qT"), (1, kT, "kT")):
                ps = pp512.tile([P, T], F32, tag="p512", name="ps512")
                for kd in range(KD):
                    nc.tensor.matmul(ps, wqkv_sb[wofs][kd][:, p * P:(p + 1) * P],
                                     hTb[kd], start=(kd == 0),
                                     stop=(kd == KD - 1))
                sb = qkp.tile([P, T], BF16, tag=f"{nmq}{p}", name=f"{nmq}{p}")
                nc.vector.tensor_copy(out=sb, in_=ps)
                if dbg is not None and b == 0:
                    nc.sync.dma_start(out=dbg[nmq][p * P:(p + 1) * P, :], in_=sb)
                dst.append(sb)

        v_aug = []
        for ts in range(NT):
            ps = pp384.tile([P, D], F32, tag="p384", name="ps384",
                            padded_shape=[P, 512])
            for kd in range(KD):
                nc.tensor.matmul(ps, hTb[kd][:, ts * P:(ts + 1) * P],
                                 wqkv_sb[2][kd], start=(kd == 0),
                                 stop=(kd == KD - 1))
            va = vp.tile([P, H, HS + 1], BF16, tag="vaug", name="vaug")
            nc.vector.memset(va[:, :, HS:HS + 1], 1.0)
            nc.vector.tensor_copy(out=va[:, :, 0:HS],
                                  in_=ps.rearrange("p (h e) -> p h e", h=H))
            v_aug.append(va)
            if dbg is not None and b == 0:
                nc.sync.dma_start(
                    out=dbg["vaug"][ts * P:(ts + 1) * P, :],
                    in_=va.rearrange("p h e -> p (h e)"))

        attnT[b] = [ap.tile([P, T], BF16, tag=f"attnT{p}", name=f"attnT{p}")
                    for p in range(NPAIR)]
        for p in range(NPAIR):
            # interleave the pair's two heads: their K=64 score matmuls sit
            # on disjoint 64-row halves of the PE array (tile_position rows
            # 0 / 64 auto-derived from the operand base partition), so
            # adjacent emission lets them execute concurrently
            pats = [ppat.tile([HS + 1, T], F32, tag="pat", name="pat")
                    for _ in (0, 1)]
            ets = ([], [])

            def emit_sc(q, ts):
                esl = slice(q * HS, (q + 1) * HS)
                ncols = T - ts * P
                psc = pp512.tile([P, T], F32, tag="p512", name="ps512")
                nc.tensor.matmul(psc[:, 0:ncols],
                                 kT[p][esl, ts * P:(ts + 1) * P],
                                 qT[p][esl, ts * P:T],
                                 start=True, stop=True)
                et = ep.tile([P, T], BF16, tag="expT", name="expT")
                nc.scalar.activation(out=et[:, 0:ncols], in_=psc[:, 0:ncols],
                                     func=mybir.ActivationFunctionType.Exp,
                                     scale=SCALE)
                nc.vector.tensor_mul(out=et[:, 0:P], in0=et[:, 0:P],
                                     in1=masku)
                ets[q].append(et)

            def emit_av(q, ts):
                ncols = T - ts * P
                nc.tensor.matmul(pats[q][:, ts * P:T],
                                 v_aug[ts][:, 2 * p + q, :],
                                 ets[q][ts][:, 0:ncols],
                                 start=(ts == 0), stop=(ts == NT - 1),
                                 skip_group_check=True)

            emit_sc(0, 0)
            emit_sc(1, 0)
            emit_sc(0, 1)
            emit_sc(1, 1)
            emit_av(0, 0)
            emit_av(1, 0)
            emit_sc(0, 2)
            emit_sc(1, 2)
            emit_av(0, 1)
            emit_av(1, 1)
            emit_sc(0, 3)
            emit_sc(1, 3)
            emit_av(0, 2)
            emit_av(1, 2)
            emit_av(0, 3)
            emit_av(1, 3)
            # normalize the pair: 1/denom from the PSUM aug row, broadcast
            # across 64 partitions on the (otherwise idle) gpsimd engine,
            # then scale the raw attn rows
            for q in (0, 1):
                # custom-DVE reciprocal can't read PSUM: stage the aug row
                # through SBUF on the scalar engine (near PSUM, has slack)
                dnm = rp.tile([1, T], F32, tag=f"dnm{q}", name=f"dnm{q}")
                nc.scalar.copy(out=dnm, in_=pats[q][HS:HS + 1, :])
                rrh = rp.tile([1, T], F32, tag=f"rrh{q}", name=f"rrh{q}")
                nc.vector.reciprocal_approx_fast(out=rrh, in_=dnm)
                if dbg is not None and b == 0 and p == 0 and q == 0:
                    nc.sync.dma_start(out=dbg["dnm"], in_=rrh)
                rbc = rp.tile([HS, T], F32, tag=f"rbc{q}", name=f"rbc{q}")
                nc.gpsimd.partition_broadcast(out_ap=rbc, in_ap=rrh)
                nc.vector.tensor_mul(out=attnT[b][p][q * HS:(q + 1) * HS, :],
                                     in0=pats[q][0:HS, :], in1=rbc)
            if dbg is not None and b == 0:
                nc.sync.dma_start(out=dbg["attnT"][p * P:(p + 1) * P, :],
                                  in_=attnT[b][p])

    def att_tail(b):
        """Wo + residual -> x2(b)."""
        x2_tiles[b] = []
        for tt in range(NT):
            po = pp384.tile([P, D], F32, tag="p384", name="ps384",
                            padded_shape=[P, 512])
            for p in range(NPAIR):
                nc.tensor.matmul(po, attnT[b][p][:, tt * P:(tt + 1) * P],
                                 wo_sb[p], start=(p == 0),
                                 stop=(p == NPAIR - 1 and zero_bias))
            if not zero_bias:
                nc.tensor.matmul(po, ones1, bo_sb, start=False, stop=True)
            x2t = x2p.tile([P, D], F32, tag="x2", name="x2")
            nc.vector.tensor_add(out=x2t, in0=po, in1=x_tiles[b][tt])
            x2_tiles[b].append(x2t)

    def mlp_back(b):
        """W1 + relu + W2 + residual + store."""
        rT = []
        for kh in range(KH):
            pm = pp512.tile([P, T], F32, tag="p512", name="ps512")
            for kd in range(KD):
                nc.tensor.matmul(pm, w1_sb[kd][:, kh * P:(kh + 1) * P],
                                 h2T[b][kd], start=(kd == 0),
                                 stop=(kd == KD - 1))
            rt = mp.tile([P, T], BF16, tag=f"rT{kh}", name=f"rT{kh}")
            nc.scalar.activation(out=rt, in_=pm,
                                 func=mybir.ActivationFunctionType.Relu,
                                 bias=bb1_sb[:, kh:kh + 1])
            rT.append(rt)
        for tt in range(NT):
            po2 = pp384.tile([P, D], F32, tag="p384", name="ps384",
                             padded_shape=[P, 512])
            for kh in range(KH):
                nc.tensor.matmul(po2, rT[kh][:, tt * P:(tt + 1) * P],
                                 w2_sb[kh], start=(kh == 0),
                                 stop=(kh == KH - 1 and zero_bias))
            if not zero_bias:
                nc.tensor.matmul(po2, ones1, bb2_sb, start=False, stop=True)
            ot = op.tile([P, D], F32, tag="ot", name="ot")
            nc.vector.tensor_add(out=ot, in0=po2, in1=x2_tiles[b][tt])
            nc.scalar.dma_start(out=out[b, tt * P:(tt + 1) * P, :], in_=ot)

    # ---------------- pipelined schedule ----------------
    load_x(0)
    load_x(1)
    pre(0)
    for b in range(nb):
        if b >= 1:
            mlp_front(b - 1)
        att_head(b)
        if b + 2 < nb:
            load_x(b + 2)
        if b + 1 < nb:
            pre(b + 1)
        if b >= 1:
            mlp_back(b - 1)
        att_tail(b)
    mlp_front(nb - 1)
    mlp_back(nb - 1)


def build(nb=NB, zero_bias=True, use_dma_transpose=True, debug=False):
    from contextlib import ExitStack

    nc = bacc.Bacc("TRN2", target_bir_lowering=False, debug=False)
    x = nc.declare_dram_parameter("x", [nb, T, D], F32, isOutput=False).ap()
    wqkv = nc.declare_dram_parameter("wqkv", [3, KD, P, H * HS], BF16,
                                     isOutput=False).ap()
    wo = nc.declare_dram_parameter("wo", [KD, P, D], BF16, isOutput=False).ap()
    w1 = nc.declare_dram_parameter("w1", [KD, P, FF], BF16, isOutput=False).ap()
    w2 = nc.declare_dram_parameter("w2", [KH, P, D], BF16, isOutput=False).ap()
    bb1 = nc.declare_dram_parameter("bb1", [P, KH], F32, isOutput=False).ap()
    bo = bb2 = None
    if not zero_bias:
        bo = nc.declare_dram_parameter("bo", [1, D], BF16, isOutput=False).ap()
        bb2 = nc.declare_dram_parameter("bb2", [1, D], BF16, isOutput=False).ap()
    out = nc.declare_dram_parameter("out", [nb, T, D], F32, isOutput=True).ap()
    dbg = None
    if debug:
        dbg = {
            "h1": nc.declare_dram_parameter("dbg_h1", [T, D], BF16, isOutput=True).ap(),
            "hT": nc.declare_dram_parameter("dbg_hT", [D, T], BF16, isOutput=True).ap(),
            "qT": nc.declare_dram_parameter("dbg_qT", [D, T], BF16, isOutput=True).ap(),
            "kT": nc.declare_dram_parameter("dbg_kT", [D, T], BF16, isOutput=True).ap(),
            "vaug": nc.declare_dram_parameter("dbg_vaug", [T, H * (HS + 1)], BF16, isOutput=True).ap(),
            "dnm": nc.declare_dram_parameter("dbg_dnm", [1, T], F32, isOutput=True).ap(),
            "attnT": nc.declare_dram_parameter("dbg_attnT", [D, T], BF16, isOutput=True).ap(),
        }

    with tile.TileContext(nc) as tc:
        with ExitStack() as ctx:
            _emit(nc, tc, ctx, x, wqkv, wo, w1, w2, bb1, out, nb=nb,
                  zero_bias=zero_bias, bo=bo, bb2=bb2,
                  use_dma_transpose=use_dma_transpose, dbg=dbg)
    nc.compile()
    return nc


def _pack_qkv(w, g1):
    # [H, D, HS] * g1[d] -> [KD, 128, H*HS]
    w = w * g1[None, :, None]
    return w.transpose(1, 0, 2).reshape(D, H * HS).reshape(KD, P, H * HS)


def run(inputs, trace=False, use_dma_transpose=True, **kw):
    bf = ml_dtypes.bfloat16
    x = np.ascontiguousarray(np.asarray(inputs["x"], dtype=np.float32))
    g1 = np.asarray(inputs["g1"], np.float32)
    b1v = np.asarray(inputs["b1"], np.float32)
    g2 = np.asarray(inputs["g2"], np.float32)
    b2v = np.asarray(inputs["b2"], np.float32)
    assert np.abs(b1v).max() == 0.0 and np.abs(b2v).max() == 0.0, \
        "nonzero LN beta not supported by this build"
    bo = np.asarray(inputs["bo"], np.float32)
    bb2 = np.asarray(inputs["bb2"], np.float32)
    zero_bias = (np.abs(bo).max() == 0.0) and (np.abs(bb2).max() == 0.0)

    wqkv = np.stack([
        _pack_qkv(np.asarray(inputs["Wq"], np.float32), g1),
        _pack_qkv(np.asarray(inputs["Wk"], np.float32), g1),
        _pack_qkv(np.asarray(inputs["Wv"], np.float32), g1),
    ]).astype(bf)
    shared = {
        "wqkv": wqkv,
        "wo": np.asarray(inputs["Wo"], np.float32)
              .reshape(KD, P, D).astype(bf),
        "w1": (np.asarray(inputs["W1"], np.float32) * g2[:, None])
              .reshape(KD, P, FF).astype(bf),
        "w2": np.asarray(inputs["W2"], np.float32)
              .reshape(KH, P, D).astype(bf),
        "bb1": np.ascontiguousarray(
            np.asarray(inputs["bb1"], np.float32).reshape(KH, P).T),
    }
    if not zero_bias:
        shared["bo"] = bo.reshape(1, D).astype(bf)
        shared["bb2"] = bb2.reshape(1, D).astype(bf)

    nc = build(zero_bias=zero_bias, use_dma_transpose=use_dma_transpose)
    in_maps = []
    for c in range(NCORES):
        m = dict(shared)
        m["x"] = np.ascontiguousarray(x[c * NB:(c + 1) * NB])
        in_maps.append(m)
    res = run_bass_kernel_spmd(nc, in_maps, list(range(NCORES)), trace=trace,
                               **kw)
    outv = np.concatenate([r["out"] for r in res.results], axis=0)
    return outv, res


def kernel(**inputs):
    return run(inputs)[0]


if __name__ == "__main__":
    print("built", build())
